# revision 12
# baseline (speedup 1.0000x reference)
"""Trainium2 Bass kernel for CausalSelfAttention (B=4, T=2048, C=2048, H=16).

Sharding: 8 cores = 4 batches x 2 head-groups (8 heads each).
Each core computes q/k/v projections for its heads, RoPE, causal attention,
and a partial output projection (row-parallel c_proj over its heads' columns).
Host sums the two partials per batch (standard row-parallel TP unshard).

On-chip layout notes:
  - All matmul contractions run with the contracted dim on partitions.
  - Host pre-transposes x and weights so every DMA is contiguous.
  - Scores are computed transposed (s^T[tk, tq]) so softmax normalization
    becomes: partition-sum via ones-matmul + reciprocal + DMA-replicate
    broadcast, and att@v needs no on-chip transposes at all.
  - RoPE rotate-half is a fixed 128x128 signed permutation applied via one
    extra matmul per q/k tile; cos/sin enter as elementwise tables.
  - Output projection is software-pipelined one chunk behind attention so
    the softmax-normalization DMA round trip never stalls the PE.
  - All biases are zero in this model; v/proj biases would be exactly
    host-addable (att rows sum to 1), q/k biases ride along in the PSUM->SBUF
    copy. Host adds bv @ Wp^T + bp to the output for generality.
"""

import numpy as np
import ml_dtypes

import concourse.bass as bass
import concourse.mybir as mybir
import concourse.tile as tile
from concourse import bacc
from concourse.alu_op_type import AluOpType
from concourse.bass import ds
from concourse.bass_utils import run_bass_kernel_spmd

BF16 = ml_dtypes.bfloat16
F32 = np.float32

B = 4
C = 2048
H = 16
D = 128
HPC = 8          # heads per core
P = 128
CH = 512         # tq chunk width
NCT = C // P     # 16 contraction tiles
AF = mybir.ActivationFunctionType
SCALE = 1.0 / float(np.sqrt(np.float32(D)))


def build_nc(T=2048):
    NCH = T // CH
    NTT = T // P
    dt = mybir.dt
    nc = bacc.Bacc(None, target_bir_lowering=False)

    NCH0 = T // CH
    xT = nc.dram_tensor("xT", [NCH0, P, NCT, CH], dt.bfloat16, kind="ExternalInput")
    wq = nc.dram_tensor("wq", [HPC, P, NCT, D], dt.bfloat16, kind="ExternalInput")
    wk = nc.dram_tensor("wk", [HPC, P, NCT, D], dt.bfloat16, kind="ExternalInput")
    wv = nc.dram_tensor("wv", [C, HPC * D], dt.bfloat16, kind="ExternalInput")
    wp = nc.dram_tensor("wp", [C // 256, P, HPC, 256], dt.bfloat16, kind="ExternalInput")
    ab_a = nc.dram_tensor("ab_a", [D, T], dt.bfloat16, kind="ExternalInput")
    ab_b = nc.dram_tensor("ab_b", [D, T], dt.bfloat16, kind="ExternalInput")
    bq = nc.dram_tensor("bq", [D, HPC], dt.float32, kind="ExternalInput")
    bk = nc.dram_tensor("bk", [D, HPC], dt.float32, kind="ExternalInput")
    maskm = nc.dram_tensor("maskm", [P, P], dt.bfloat16, kind="ExternalInput")
    pt = nc.dram_tensor("pt", [D, D], dt.bfloat16, kind="ExternalInput")
    onc = nc.dram_tensor("onc", [P, 1], dt.bfloat16, kind="ExternalInput")
    out = nc.dram_tensor("out", [T, C], dt.float32, kind="ExternalOutput")
    scratch = nc.dram_tensor("den_scratch", [NCH, HPC, CH], dt.float32)

    wv_r = wv.rearrange("(ct p) d -> p ct d", p=P)

    with tile.TileContext(nc) as tc:
        with (
            tc.tile_pool(name="consts", bufs=1) as consts,
            tc.tile_pool(name="keep", bufs=1) as keep,
        ):
            mask_sb = consts.tile([P, P], dt.bfloat16)
            pt_sb = consts.tile([D, D], dt.bfloat16)
            bq_sb = consts.tile([D, HPC], dt.float32)
            bk_sb = consts.tile([D, HPC], dt.float32)
            onc_sb = consts.tile([P, 1], dt.bfloat16)

            def load_consts():
                nc.sync.dma_start(out=mask_sb, in_=maskm[:])
                nc.sync.dma_start(out=pt_sb, in_=pt[:])
                nc.sync.dma_start(out=bq_sb, in_=bq[:])
                nc.sync.dma_start(out=bk_sb, in_=bk[:])
                nc.sync.dma_start(out=onc_sb, in_=onc[:])

            kT = keep.tile([P, HPC, T], dt.bfloat16)
            # v in [t, d] layout: [t%128, half, t-tile, 4 heads x D]
            vS = keep.tile([P, 2, NTT, CH], dt.bfloat16)
            wv_sb = keep.tile([P, NCT, 2 * CH], dt.bfloat16)

            with (
                tc.tile_pool(name="xw", bufs=1) as xwp,
                tc.tile_pool(name="wtp", bufs=3) as wtp,
                tc.tile_pool(name="work", bufs=4) as work,
                tc.tile_pool(name="qpp", bufs=8) as qpp,
                tc.tile_pool(name="denp", bufs=2) as denp,
                tc.tile_pool(name="ytp", bufs=2) as ytp,
                tc.tile_pool(name="wpp", bufs=4) as wpp,
                tc.tile_pool(name="outp", bufs=2) as outp,
                tc.tile_pool(name="ps_a", bufs=4, space="PSUM") as ps_a,
                tc.tile_pool(name="ps_s", bufs=4, space="PSUM") as ps_s,
            ):
                OC = 256

                def outproj_units(yTc, j):
                    for oc in range(C // OC):
                        wps = wpp.tile([P, HPC, OC], dt.bfloat16, tag="wps")
                        nc.sync.dma_start(out=wps, in_=wp[oc])
                        for tt in range(CH // P):
                            ps = ps_a.tile([P, CH], dt.float32, tag="a")
                            for hc in range(HPC):
                                nc.tensor.matmul(
                                    ps[:, ds(0, OC)],
                                    lhsT=yTc[:, hc, ds(tt * D, D)],
                                    rhs=wps[:, hc, :],
                                    start=(hc == 0),
                                    stop=(hc == HPC - 1),
                                )
                            ot = outp.tile([P, OC], dt.float32, tag="ot")
                            nc.vector.tensor_copy(out=ot, in_=ps[:, ds(0, OC)])
                            nc.scalar.dma_start(
                                out=out[
                                    ds((4 * j + tt) * P, P), ds(oc * OC, OC)
                                ],
                                in_=ot,
                            )
                            yield None

                PULLS = [6, 6, 5, 5, 4, 3, 2, 1]
                prev_gen = None
                wt_pre = {}
                for j in range(NCH):
                    cols = ds(j * CH, CH)
                    if j == 0:
                        load_consts()
                        for h in range(2):
                            wt = wtp.tile([P, NCT, D], dt.bfloat16, tag="wt")
                            nc.sync.dma_start(out=wt, in_=wq[h])
                            wt_pre[h] = wt
                    xc = xwp.tile([P, NCT, CH], dt.bfloat16, tag="xc")
                    for cg in range(4):
                        nc.sync.dma_start(
                            out=xc[:, ds(cg * 4, 4), :],
                            in_=xT[j][:, ds(cg * 4, 4), :],
                        )
                    a_sb = work.tile([D, CH], dt.bfloat16, tag="abA", bufs=2)
                    nc.scalar.dma_start(out=a_sb, in_=ab_a[:, cols])
                    b_sb = work.tile([D, CH], dt.bfloat16, tag="abB", bufs=2)
                    nc.scalar.dma_start(out=b_sb, in_=ab_b[:, cols])

                    qp_tiles = []

                    def emit_rope(raw, dest):
                        # q'/k' = A (.) raw + B (.) (P @ raw), via one PE
                        # matmul for the rotate-half permutation
                        rps = ps_a.tile([P, CH], dt.float32, tag="a")
                        nc.tensor.matmul(
                            rps, lhsT=pt_sb, rhs=raw, start=True, stop=True
                        )
                        t1 = work.tile([P, CH], dt.bfloat16, tag="t1", bufs=2)
                        nc.gpsimd.tensor_tensor(
                            out=t1, in0=raw, in1=a_sb, op=AluOpType.mult
                        )
                        t2 = work.tile([P, CH], dt.bfloat16, tag="t2", bufs=2)
                        nc.vector.tensor_tensor(
                            out=t2, in0=rps, in1=b_sb, op=AluOpType.mult
                        )
                        nc.gpsimd.tensor_tensor(
                            out=dest, in0=t1, in1=t2, op=AluOpType.add
                        )

                    pending = None  # one-deep pipeline so rot never stalls PE
                    for qk in range(2):
                        wsrc = wq if qk == 0 else wk
                        bsrc = bq_sb if qk == 0 else bk_sb
                        for h in range(HPC):
                            if j == 0 and qk == 0 and h in wt_pre:
                                wt = wt_pre.pop(h)
                            else:
                                wt = wtp.tile([P, NCT, D], dt.bfloat16, tag="wt")
                                nc.sync.dma_start(out=wt, in_=wsrc[h])
                            ps = ps_a.tile([P, CH], dt.float32, tag="a")
                            for ct in range(NCT):
                                nc.tensor.matmul(
                                    ps,
                                    lhsT=wt[:, ct, :],
                                    rhs=xc[:, ct, :],
                                    start=(ct == 0),
                                    stop=(ct == NCT - 1),
                                )
                            raw = work.tile([P, CH], dt.bfloat16, tag="raw", bufs=3)
                            nc.vector.tensor_tensor(
                                out=raw,
                                in0=ps,
                                in1=bsrc[:, ds(h, 1)].to_broadcast([P, CH]),
                                op=AluOpType.add,
                            )
                            if j == 0 and qk == 0 and 2 <= h < 6:
                                qtr = h - 2
                                nc.sync.dma_start(
                                    out=wv_sb[:, :, ds(qtr * 256, 256)],
                                    in_=wv_r[:, :, ds(qtr * 256, 256)],
                                )
                            if qk == 0:
                                dest = qpp.tile([P, CH], dt.bfloat16, tag="qp")
                                qp_tiles.append(dest)
                            else:
                                dest = kT[:, h, cols]
                            if pending is not None:
                                emit_rope(*pending)
                            pending = (raw, dest)
                    emit_rope(*pending)

                    # v projection: x-stationary, v comes out in [t, d] layout
                    for half in range(2):
                        for tt in range(4):
                            ps = ps_a.tile([P, CH], dt.float32, tag="a")
                            for ct in range(NCT):
                                nc.tensor.matmul(
                                    ps,
                                    lhsT=xc[:, ct, ds(tt * D, D)],
                                    rhs=wv_sb[:, ct, ds(half * CH, CH)],
                                    start=(ct == 0),
                                    stop=(ct == NCT - 1),
                                )
                            nc.scalar.activation(
                                vS[:, half, 4 * j + tt, :], ps, AF.Copy
                            )

                    yTc = ytp.tile([P, HPC, CH], dt.bfloat16, tag="ytc")
                    for h in range(HPC):
                        qp = qp_tiles[h]
                        half, hh = h // 4, h % 4
                        den_a = denp.tile([P, CH], dt.bfloat16, tag="dena")
                        yps = ps_a.tile([P, CH], dt.float32, tag="a")
                        ntk = 4 * (j + 1)
                        exq = []  # (ex, i, off) pending y-matmuls
                        for i in range(ntk):
                            sps = ps_s.tile([P, CH], dt.float32, tag="s", bufs=4)
                            m = i - 4 * j
                            off = max(m, 0) * D  # valid tq cols start here
                            w = CH - off
                            nc.tensor.matmul(
                                sps[:, ds(off, w)],
                                lhsT=kT[:, h, ds(i * D, D)],
                                rhs=qp[:, ds(off, w)],
                                start=True,
                                stop=True,
                            )
                            ex = work.tile([P, CH], dt.bfloat16, tag="ex", bufs=4)
                            nc.scalar.activation(
                                ex[:, ds(off, w)], sps[:, ds(off, w)],
                                AF.Exp, scale=SCALE,
                            )
                            if m >= 0:
                                # triangular mask on the diagonal 128-block
                                nc.vector.tensor_tensor(
                                    out=ex[:, ds(off, D)],
                                    in0=ex[:, ds(off, D)],
                                    in1=mask_sb,
                                    op=AluOpType.mult,
                                )
                            if i == 0:
                                nc.vector.tensor_copy(
                                    out=den_a[:, ds(off, w)], in_=ex[:, ds(off, w)]
                                )
                                if off > 0:
                                    nc.vector.memset(den_a[:, ds(0, off)], 0.0)
                            else:
                                nc.vector.tensor_tensor(
                                    out=den_a[:, ds(off, w)],
                                    in0=den_a[:, ds(off, w)],
                                    in1=ex[:, ds(off, w)],
                                    op=AluOpType.add,
                                )
                            exq.append((ex, i, off))
                            if len(exq) > 3:
                                pex, pi, poff = exq.pop(0)
                                nc.tensor.matmul(
                                    yps[:, ds(poff, CH - poff)],
                                    lhsT=vS[:, half, pi, ds(hh * D, D)],
                                    rhs=pex[:, ds(poff, CH - poff)],
                                    start=(pi == 0),
                                    stop=False,
                                )
                        while exq:
                            pex, pi, poff = exq.pop(0)
                            nc.tensor.matmul(
                                yps[:, ds(poff, CH - poff)],
                                lhsT=vS[:, half, pi, ds(hh * D, D)],
                                rhs=pex[:, ds(poff, CH - poff)],
                                start=(pi == 0),
                                stop=(not exq),
                            )
                        yraw = work.tile([P, CH], dt.bfloat16, tag="yraw", bufs=4)
                        nc.vector.tensor_copy(out=yraw, in_=yps)
                        dps = ps_a.tile([P, CH], dt.float32, tag="a")
                        dsum = dps[ds(0, 1), :]
                        nc.tensor.matmul(
                            dsum, lhsT=onc_sb, rhs=den_a, start=True, stop=True
                        )
                        # per-head normalization: reciprocal, DRAM round trip
                        # for the partition broadcast, multiply into yTc
                        rec_h = denp.tile([1, CH], dt.float32, tag="rec")
                        nc.vector.reciprocal(rec_h, dsum)
                        nc.gpsimd.dma_start(out=scratch[j, h], in_=rec_h)
                        rbc = work.tile([P, CH], dt.float32, tag="rbc", bufs=2)
                        nc.gpsimd.dma_start(
                            out=rbc,
                            in_=scratch[j, h][None, :].to_broadcast([P, CH]),
                        )
                        nc.gpsimd.tensor_tensor(
                            out=yTc[:, h, :],
                            in0=yraw,
                            in1=rbc,
                            op=AluOpType.mult,
                        )
                        if prev_gen is not None:
                            for _ in range(PULLS[h]):
                                if next(prev_gen, StopIteration) is StopIteration:
                                    break
                    prev_gen = outproj_units(yTc, j)
                for _ in prev_gen:
                    pass
    nc.compile()
    return nc


def _rope_tables(T):
    inv_freq = (
        1.0 / (10000.0 ** (np.arange(0, D, 2, dtype=np.float32) / np.float32(D)))
    ).astype(np.float32)
    t = np.arange(T, dtype=np.float32)
    freqs = t[:, None] * inv_freq[None, :]
    emb = np.concatenate((freqs, freqs), axis=-1)
    cos = np.cos(emb).astype(np.float32)
    sin = np.sin(emb).astype(np.float32)
    A = np.ascontiguousarray((cos + sin).T).astype(BF16)
    Bt = np.ascontiguousarray((cos - sin).T).astype(BF16)
    return A, Bt


def _rot_pt():
    Pm = np.zeros((D, D), dtype=np.float32)
    for d in range(64):
        Pm[d, 2 * d + 1] = -1.0
        Pm[64 + d, 2 * d] = 1.0
    return np.ascontiguousarray(Pm.T).astype(BF16)


def _maskm():
    # maskm[p, c] = 0 where tq < tk within a diagonal 128x128 block
    row = np.arange(P)[:, None]
    col = np.arange(P)[None, :]
    return np.where(col < row, 0.0, 1.0).astype(BF16)


def _xtile(xb, T):
    # [T, C] -> [chunk, p, ct, CH] with each chunk tile contiguous
    a = np.ascontiguousarray(xb.T)  # [C, T]
    return np.ascontiguousarray(
        a.reshape(NCT, P, T // CH, CH).transpose(2, 1, 0, 3)
    ).astype(BF16)


def _wtile(w):
    # [1024, C] -> [h, p, ct, D] with each head tile contiguous
    a = np.ascontiguousarray(w.T)  # [C, 1024]
    return np.ascontiguousarray(
        a.reshape(NCT, P, HPC, D).transpose(2, 1, 0, 3)
    ).astype(BF16)


def _ptile(w):
    # [C, 1024] -> [oc, p, hc, 256] with each oc tile contiguous
    a = np.ascontiguousarray(w.T)  # [1024, C]
    return np.ascontiguousarray(
        a.reshape(HPC, P, C // 256, 256).transpose(2, 1, 0, 3)
    ).astype(BF16)


def make_in_maps(x, w_attn, b_attn, w_proj, b_proj, T=2048):
    A, Bt = _rope_tables(T)
    pt = _rot_pt()
    maskm = _maskm()
    onc = np.ones((P, 1), dtype=BF16)
    in_maps = []
    for core in range(8):
        b, g = core // 2, core % 2
        gs = slice(g * 1024, (g + 1) * 1024)
        in_maps.append(
            {
                "xT": _xtile(x[b][:T], T),
                "wq": _wtile(w_attn[gs, :]),
                "wk": _wtile(w_attn[2048:4096][gs]),
                "wv": np.ascontiguousarray(w_attn[4096:6144][gs, :].T).astype(BF16),
                "wp": _ptile(w_proj[:, gs]),
                "ab_a": A,
                "ab_b": Bt,
                "bq": np.ascontiguousarray(
                    b_attn[gs].reshape(HPC, D).T
                ).astype(np.float32),
                "bk": np.ascontiguousarray(
                    b_attn[2048:4096][gs].reshape(HPC, D).T
                ).astype(np.float32),
                "maskm": maskm,
                "pt": pt,
                "onc": onc,
            }
        )
    return in_maps


_NC_CACHE = {}


def run(x, w_attn, b_attn, w_proj, b_proj, trace=False, trace_cores=None):
    x = np.asarray(x, dtype=np.float32)
    w_attn = np.asarray(w_attn, dtype=np.float32)
    b_attn = np.asarray(b_attn, dtype=np.float32)
    w_proj = np.asarray(w_proj, dtype=np.float32)
    b_proj = np.asarray(b_proj, dtype=np.float32)
    T = x.shape[1]
    if T not in _NC_CACHE:
        _NC_CACHE[T] = build_nc(T)
    nc = _NC_CACHE[T]
    in_maps = make_in_maps(x, w_attn, b_attn, w_proj, b_proj, T=T)
    res = run_bass_kernel_spmd(
        nc, in_maps, core_ids=list(range(8)), trace=trace, trace_cores=trace_cores
    )
    out = np.zeros((B, T, C), dtype=np.float32)
    for b in range(B):
        out[b] = res.results[2 * b]["out"] + res.results[2 * b + 1]["out"]
    # v-bias is exactly additive post-attention (att rows sum to 1), so it and
    # the proj bias are applied on host; both are zero for this model.
    bv = b_attn[2 * C : 3 * C]
    if bv.any() or b_proj.any():
        out += (bv.astype(np.float64) @ w_proj.T.astype(np.float64)).astype(
            np.float32
        ) + b_proj
    return out, res


def kernel(x, w_attn, b_attn, w_proj, b_proj):
    out, _ = run(x, w_attn, b_attn, w_proj, b_proj, trace=False)
    return out


# revision 13
# speedup vs baseline: 1.0042x; 1.0042x over previous
"""Trainium2 Bass kernel for CausalSelfAttention (B=4, T=2048, C=2048, H=16).

Sharding: 8 cores = 4 batches x 2 head-groups (8 heads each).
Each core computes q/k/v projections for its heads, RoPE, causal attention,
and a partial output projection (row-parallel c_proj over its heads' columns).
Host sums the two partials per batch (standard row-parallel TP unshard).

On-chip layout notes:
  - All matmul contractions run with the contracted dim on partitions.
  - Host pre-transposes x and weights so every DMA is contiguous.
  - Scores are computed transposed (s^T[tk, tq]) so softmax normalization
    becomes: partition-sum via ones-matmul + reciprocal + DMA-replicate
    broadcast, and att@v needs no on-chip transposes at all.
  - RoPE rotate-half is a fixed 128x128 signed permutation applied via one
    extra matmul per q/k tile; cos/sin enter as elementwise tables.
  - Output projection is software-pipelined one chunk behind attention so
    the softmax-normalization DMA round trip never stalls the PE.
  - All biases are zero in this model; v/proj biases would be exactly
    host-addable (att rows sum to 1), q/k biases ride along in the PSUM->SBUF
    copy. Host adds bv @ Wp^T + bp to the output for generality.
"""

import numpy as np
import ml_dtypes

import concourse.bass as bass
import concourse.mybir as mybir
import concourse.tile as tile
from concourse import bacc
from concourse.alu_op_type import AluOpType
from concourse.bass import ds
from concourse.bass_utils import run_bass_kernel_spmd

BF16 = ml_dtypes.bfloat16
F32 = np.float32

B = 4
C = 2048
H = 16
D = 128
HPC = 8          # heads per core
P = 128
CH = 512         # tq chunk width
NCT = C // P     # 16 contraction tiles
AF = mybir.ActivationFunctionType
SCALE = 1.0 / float(np.sqrt(np.float32(D)))


def build_nc(T=2048):
    NCH = T // CH
    NTT = T // P
    dt = mybir.dt
    nc = bacc.Bacc(None, target_bir_lowering=False)

    NCH0 = T // CH
    xT = nc.dram_tensor("xT", [NCH0, P, NCT, CH], dt.bfloat16, kind="ExternalInput")
    wq = nc.dram_tensor("wq", [HPC, P, NCT, D], dt.bfloat16, kind="ExternalInput")
    wk = nc.dram_tensor("wk", [HPC, P, NCT, D], dt.bfloat16, kind="ExternalInput")
    wv = nc.dram_tensor("wv", [C, HPC * D], dt.bfloat16, kind="ExternalInput")
    wp = nc.dram_tensor("wp", [C // 256, P, HPC, 256], dt.bfloat16, kind="ExternalInput")
    ab_a = nc.dram_tensor("ab_a", [D, T], dt.bfloat16, kind="ExternalInput")
    ab_b = nc.dram_tensor("ab_b", [D, T], dt.bfloat16, kind="ExternalInput")
    bq = nc.dram_tensor("bq", [D, HPC], dt.float32, kind="ExternalInput")
    bk = nc.dram_tensor("bk", [D, HPC], dt.float32, kind="ExternalInput")
    maskm = nc.dram_tensor("maskm", [P, P], dt.bfloat16, kind="ExternalInput")
    pt = nc.dram_tensor("pt", [D, D], dt.bfloat16, kind="ExternalInput")
    onc = nc.dram_tensor("onc", [P, 1], dt.bfloat16, kind="ExternalInput")
    out = nc.dram_tensor("out", [T, C], dt.float32, kind="ExternalOutput")
    scratch = nc.dram_tensor("den_scratch", [NCH, HPC, CH], dt.float32)

    wv_r = wv.rearrange("(ct p) d -> p ct d", p=P)

    with tile.TileContext(nc) as tc:
        with (
            tc.tile_pool(name="consts", bufs=1) as consts,
            tc.tile_pool(name="keep", bufs=1) as keep,
        ):
            mask_sb = consts.tile([P, P], dt.bfloat16)
            pt_sb = consts.tile([D, D], dt.bfloat16)
            bq_sb = consts.tile([D, HPC], dt.float32)
            bk_sb = consts.tile([D, HPC], dt.float32)
            onc_sb = consts.tile([P, 1], dt.bfloat16)

            def load_consts():
                nc.sync.dma_start(out=mask_sb, in_=maskm[:])
                nc.sync.dma_start(out=pt_sb, in_=pt[:])
                nc.sync.dma_start(out=bq_sb, in_=bq[:])
                nc.sync.dma_start(out=bk_sb, in_=bk[:])
                nc.sync.dma_start(out=onc_sb, in_=onc[:])

            kT = keep.tile([P, HPC, T], dt.bfloat16)
            # v in [t, d] layout: [t%128, half, t-tile, 4 heads x D]
            vS = keep.tile([P, 2, NTT, CH], dt.bfloat16)
            wv_sb = keep.tile([P, NCT, 2 * CH], dt.bfloat16)

            with (
                tc.tile_pool(name="xw", bufs=1) as xwp,
                tc.tile_pool(name="wtp", bufs=3) as wtp,
                tc.tile_pool(name="work", bufs=4) as work,
                tc.tile_pool(name="qpp", bufs=8) as qpp,
                tc.tile_pool(name="denp", bufs=2) as denp,
                tc.tile_pool(name="ytp", bufs=2) as ytp,
                tc.tile_pool(name="wpp", bufs=4) as wpp,
                tc.tile_pool(name="outp", bufs=2) as outp,
                tc.tile_pool(name="ps_a", bufs=4, space="PSUM") as ps_a,
                tc.tile_pool(name="ps_s", bufs=4, space="PSUM") as ps_s,
            ):
                OC = 256

                def outproj_units(yTc, j):
                    for oc in range(C // OC):
                        wps = wpp.tile([P, HPC, OC], dt.bfloat16, tag="wps")
                        nc.sync.dma_start(out=wps, in_=wp[oc])
                        for tt in range(CH // P):
                            ps = ps_a.tile([P, CH], dt.float32, tag="a")
                            for hc in range(HPC):
                                nc.tensor.matmul(
                                    ps[:, ds(0, OC)],
                                    lhsT=yTc[:, hc, ds(tt * D, D)],
                                    rhs=wps[:, hc, :],
                                    start=(hc == 0),
                                    stop=(hc == HPC - 1),
                                )
                            ot = outp.tile([P, OC], dt.float32, tag="ot")
                            nc.vector.tensor_copy(out=ot, in_=ps[:, ds(0, OC)])
                            nc.scalar.dma_start(
                                out=out[
                                    ds((4 * j + tt) * P, P), ds(oc * OC, OC)
                                ],
                                in_=ot,
                            )
                            yield None

                PULLS = [6, 6, 5, 5, 4, 3, 2, 1]
                prev_gen = None
                wt_pre = {}
                for j in range(NCH):
                    cols = ds(j * CH, CH)
                    if j == 0:
                        load_consts()
                        for h in range(2):
                            wt = wtp.tile([P, NCT, D], dt.bfloat16, tag="wt")
                            nc.sync.dma_start(out=wt, in_=wq[h])
                            wt_pre[h] = wt
                    xc = xwp.tile([P, NCT, CH], dt.bfloat16, tag="xc")
                    for cg in range(4):
                        nc.sync.dma_start(
                            out=xc[:, ds(cg * 4, 4), :],
                            in_=xT[j][:, ds(cg * 4, 4), :],
                        )
                    a_sb = work.tile([D, CH], dt.bfloat16, tag="abA", bufs=2)
                    nc.scalar.dma_start(out=a_sb, in_=ab_a[:, cols])
                    b_sb = work.tile([D, CH], dt.bfloat16, tag="abB", bufs=2)
                    nc.scalar.dma_start(out=b_sb, in_=ab_b[:, cols])

                    qp_tiles = []

                    def emit_rope(raw, dest):
                        # q'/k' = A (.) raw + B (.) (P @ raw), via one PE
                        # matmul for the rotate-half permutation
                        rps = ps_a.tile([P, CH], dt.float32, tag="a")
                        nc.tensor.matmul(
                            rps, lhsT=pt_sb, rhs=raw, start=True, stop=True
                        )
                        t1 = work.tile([P, CH], dt.bfloat16, tag="t1", bufs=2)
                        nc.gpsimd.tensor_tensor(
                            out=t1, in0=raw, in1=a_sb, op=AluOpType.mult
                        )
                        t2 = work.tile([P, CH], dt.bfloat16, tag="t2", bufs=2)
                        nc.vector.tensor_tensor(
                            out=t2, in0=rps, in1=b_sb, op=AluOpType.mult
                        )
                        nc.gpsimd.tensor_tensor(
                            out=dest, in0=t1, in1=t2, op=AluOpType.add
                        )

                    pending = None  # one-deep pipeline so rot never stalls PE
                    for qk in range(2):
                        wsrc = wq if qk == 0 else wk
                        bsrc = bq_sb if qk == 0 else bk_sb
                        for h in range(HPC):
                            if j == 0 and qk == 0 and h in wt_pre:
                                wt = wt_pre.pop(h)
                            else:
                                wt = wtp.tile([P, NCT, D], dt.bfloat16, tag="wt")
                                nc.sync.dma_start(out=wt, in_=wsrc[h])
                            ps = ps_a.tile([P, CH], dt.float32, tag="a")
                            for ct in range(NCT):
                                nc.tensor.matmul(
                                    ps,
                                    lhsT=wt[:, ct, :],
                                    rhs=xc[:, ct, :],
                                    start=(ct == 0),
                                    stop=(ct == NCT - 1),
                                )
                            raw = work.tile([P, CH], dt.bfloat16, tag="raw", bufs=3)
                            nc.vector.tensor_tensor(
                                out=raw,
                                in0=ps,
                                in1=bsrc[:, ds(h, 1)].to_broadcast([P, CH]),
                                op=AluOpType.add,
                            )
                            if j == 0 and qk == 0 and 2 <= h < 6:
                                qtr = h - 2
                                nc.sync.dma_start(
                                    out=wv_sb[:, :, ds(qtr * 256, 256)],
                                    in_=wv_r[:, :, ds(qtr * 256, 256)],
                                )
                            if qk == 0:
                                dest = qpp.tile([P, CH], dt.bfloat16, tag="qp")
                                qp_tiles.append(dest)
                            else:
                                dest = kT[:, h, cols]
                            if pending is not None:
                                emit_rope(*pending)
                            pending = (raw, dest)
                    emit_rope(*pending)

                    # v projection: x-stationary, v comes out in [t, d] layout
                    for half in range(2):
                        for tt in range(4):
                            ps = ps_a.tile([P, CH], dt.float32, tag="a")
                            for ct in range(NCT):
                                nc.tensor.matmul(
                                    ps,
                                    lhsT=xc[:, ct, ds(tt * D, D)],
                                    rhs=wv_sb[:, ct, ds(half * CH, CH)],
                                    start=(ct == 0),
                                    stop=(ct == NCT - 1),
                                )
                            nc.scalar.activation(
                                vS[:, half, 4 * j + tt, :], ps, AF.Copy
                            )

                    yTc = ytp.tile([P, HPC, CH], dt.bfloat16, tag="ytc")
                    for h in range(HPC):
                        qp = qp_tiles[h]
                        half, hh = h // 4, h % 4
                        den_a = denp.tile([P, CH], dt.bfloat16, tag="dena")
                        yps = ps_a.tile([P, CH], dt.float32, tag="a")
                        ntk = 4 * (j + 1)
                        exq = []  # (ex, i, off) pending y-matmuls
                        for i in range(ntk):
                            sps = ps_s.tile([P, CH], dt.float32, tag="s", bufs=4)
                            m = i - 4 * j
                            off = max(m, 0) * D  # valid tq cols start here
                            w = CH - off
                            nc.tensor.matmul(
                                sps[:, ds(off, w)],
                                lhsT=kT[:, h, ds(i * D, D)],
                                rhs=qp[:, ds(off, w)],
                                start=True,
                                stop=True,
                            )
                            ex = work.tile([P, CH], dt.bfloat16, tag="ex", bufs=4)
                            nc.scalar.activation(
                                ex[:, ds(off, w)], sps[:, ds(off, w)],
                                AF.Exp, scale=SCALE,
                            )
                            if m >= 0:
                                # triangular mask on the diagonal 128-block
                                nc.vector.tensor_tensor(
                                    out=ex[:, ds(off, D)],
                                    in0=ex[:, ds(off, D)],
                                    in1=mask_sb,
                                    op=AluOpType.mult,
                                )
                            if i == 0:
                                nc.vector.tensor_copy(
                                    out=den_a[:, ds(off, w)], in_=ex[:, ds(off, w)]
                                )
                                if off > 0:
                                    nc.vector.memset(den_a[:, ds(0, off)], 0.0)
                            else:
                                nc.vector.tensor_tensor(
                                    out=den_a[:, ds(off, w)],
                                    in0=den_a[:, ds(off, w)],
                                    in1=ex[:, ds(off, w)],
                                    op=AluOpType.add,
                                )
                            exq.append((ex, i, off))
                            if len(exq) > 3:
                                pex, pi, poff = exq.pop(0)
                                nc.tensor.matmul(
                                    yps[:, ds(poff, CH - poff)],
                                    lhsT=vS[:, half, pi, ds(hh * D, D)],
                                    rhs=pex[:, ds(poff, CH - poff)],
                                    start=(pi == 0),
                                    stop=False,
                                )
                        while exq:
                            pex, pi, poff = exq.pop(0)
                            nc.tensor.matmul(
                                yps[:, ds(poff, CH - poff)],
                                lhsT=vS[:, half, pi, ds(hh * D, D)],
                                rhs=pex[:, ds(poff, CH - poff)],
                                start=(pi == 0),
                                stop=(not exq),
                            )
                        yraw = work.tile([P, CH], dt.bfloat16, tag="yraw", bufs=4)
                        nc.vector.tensor_copy(out=yraw, in_=yps)
                        dps = ps_a.tile([P, CH], dt.float32, tag="a")
                        dsum = dps[ds(0, 1), :]
                        nc.tensor.matmul(
                            dsum, lhsT=onc_sb, rhs=den_a, start=True, stop=True
                        )
                        # per-head normalization: reciprocal, DRAM round trip
                        # for the partition broadcast, multiply into yTc
                        rec_h = denp.tile([1, CH], dt.float32, tag="rec")
                        nc.vector.reciprocal(rec_h, dsum)
                        nc.sync.dma_start(out=scratch[j, h], in_=rec_h)
                        rbc = work.tile([P, CH], dt.float32, tag="rbc", bufs=2)
                        nc.sync.dma_start(
                            out=rbc,
                            in_=scratch[j, h][None, :].to_broadcast([P, CH]),
                        )
                        nc.gpsimd.tensor_tensor(
                            out=yTc[:, h, :],
                            in0=yraw,
                            in1=rbc,
                            op=AluOpType.mult,
                        )
                        if prev_gen is not None:
                            for _ in range(PULLS[h]):
                                if next(prev_gen, StopIteration) is StopIteration:
                                    break
                    prev_gen = outproj_units(yTc, j)
                for _ in prev_gen:
                    pass
    nc.compile()
    return nc


def _rope_tables(T):
    inv_freq = (
        1.0 / (10000.0 ** (np.arange(0, D, 2, dtype=np.float32) / np.float32(D)))
    ).astype(np.float32)
    t = np.arange(T, dtype=np.float32)
    freqs = t[:, None] * inv_freq[None, :]
    emb = np.concatenate((freqs, freqs), axis=-1)
    cos = np.cos(emb).astype(np.float32)
    sin = np.sin(emb).astype(np.float32)
    A = np.ascontiguousarray((cos + sin).T).astype(BF16)
    Bt = np.ascontiguousarray((cos - sin).T).astype(BF16)
    return A, Bt


def _rot_pt():
    Pm = np.zeros((D, D), dtype=np.float32)
    for d in range(64):
        Pm[d, 2 * d + 1] = -1.0
        Pm[64 + d, 2 * d] = 1.0
    return np.ascontiguousarray(Pm.T).astype(BF16)


def _maskm():
    # maskm[p, c] = 0 where tq < tk within a diagonal 128x128 block
    row = np.arange(P)[:, None]
    col = np.arange(P)[None, :]
    return np.where(col < row, 0.0, 1.0).astype(BF16)


def _xtile(xb, T):
    # [T, C] -> [chunk, p, ct, CH] with each chunk tile contiguous
    a = np.ascontiguousarray(xb.T)  # [C, T]
    return np.ascontiguousarray(
        a.reshape(NCT, P, T // CH, CH).transpose(2, 1, 0, 3)
    ).astype(BF16)


def _wtile(w):
    # [1024, C] -> [h, p, ct, D] with each head tile contiguous
    a = np.ascontiguousarray(w.T)  # [C, 1024]
    return np.ascontiguousarray(
        a.reshape(NCT, P, HPC, D).transpose(2, 1, 0, 3)
    ).astype(BF16)


def _ptile(w):
    # [C, 1024] -> [oc, p, hc, 256] with each oc tile contiguous
    a = np.ascontiguousarray(w.T)  # [1024, C]
    return np.ascontiguousarray(
        a.reshape(HPC, P, C // 256, 256).transpose(2, 1, 0, 3)
    ).astype(BF16)


def make_in_maps(x, w_attn, b_attn, w_proj, b_proj, T=2048):
    A, Bt = _rope_tables(T)
    pt = _rot_pt()
    maskm = _maskm()
    onc = np.ones((P, 1), dtype=BF16)
    in_maps = []
    for core in range(8):
        b, g = core // 2, core % 2
        gs = slice(g * 1024, (g + 1) * 1024)
        in_maps.append(
            {
                "xT": _xtile(x[b][:T], T),
                "wq": _wtile(w_attn[gs, :]),
                "wk": _wtile(w_attn[2048:4096][gs]),
                "wv": np.ascontiguousarray(w_attn[4096:6144][gs, :].T).astype(BF16),
                "wp": _ptile(w_proj[:, gs]),
                "ab_a": A,
                "ab_b": Bt,
                "bq": np.ascontiguousarray(
                    b_attn[gs].reshape(HPC, D).T
                ).astype(np.float32),
                "bk": np.ascontiguousarray(
                    b_attn[2048:4096][gs].reshape(HPC, D).T
                ).astype(np.float32),
                "maskm": maskm,
                "pt": pt,
                "onc": onc,
            }
        )
    return in_maps


_NC_CACHE = {}


def run(x, w_attn, b_attn, w_proj, b_proj, trace=False, trace_cores=None):
    x = np.asarray(x, dtype=np.float32)
    w_attn = np.asarray(w_attn, dtype=np.float32)
    b_attn = np.asarray(b_attn, dtype=np.float32)
    w_proj = np.asarray(w_proj, dtype=np.float32)
    b_proj = np.asarray(b_proj, dtype=np.float32)
    T = x.shape[1]
    if T not in _NC_CACHE:
        _NC_CACHE[T] = build_nc(T)
    nc = _NC_CACHE[T]
    in_maps = make_in_maps(x, w_attn, b_attn, w_proj, b_proj, T=T)
    res = run_bass_kernel_spmd(
        nc, in_maps, core_ids=list(range(8)), trace=trace, trace_cores=trace_cores
    )
    out = np.zeros((B, T, C), dtype=np.float32)
    for b in range(B):
        out[b] = res.results[2 * b]["out"] + res.results[2 * b + 1]["out"]
    # v-bias is exactly additive post-attention (att rows sum to 1), so it and
    # the proj bias are applied on host; both are zero for this model.
    bv = b_attn[2 * C : 3 * C]
    if bv.any() or b_proj.any():
        out += (bv.astype(np.float64) @ w_proj.T.astype(np.float64)).astype(
            np.float32
        ) + b_proj
    return out, res


def kernel(x, w_attn, b_attn, w_proj, b_proj):
    out, _ = run(x, w_attn, b_attn, w_proj, b_proj, trace=False)
    return out
